# revision 1
# baseline (speedup 1.0000x reference)
"""C3DLoss kernel for Trainium2 — 8-core batch-parallel, raw-Bass implementation.

Per core = one batch frame b (tgt pairing partner tb = b^1):
    partial = sum over both terms (same-frame, cross-frame), all 25 shifts
              delta in [-2,2]^2, all pixels p of
        mref(p) * mq(p+delta) * exp(-50*(|xyz_r(p)-xyz_q(p+d)|^2
                                         + |rgb_r(p)-rgb_q(p+d)|^2))
    loss = -(sum of partials) / max(sum(depth_gt_mask), 1)

Device mapping:
  - Host pre-blocks every plane into G=32 W-blocks of width WB with a +-2
    halo in both dims (zero padded).  Partitions = (channel, block); dy/dx
    shifts become pure free-dim offsets, so all 25 shifts read the same
    SBUF tiles.  Host also precomputes the feature planes (xyz = xy1*depth,
    txyz = R*xyz + t, mask channels); that is <2% of the FLOPs.
  - Channels split across two tiles (PE contraction K <= 128):
      A: x, y, z, 20*(1-mq) query-mask channel (ref side 0) -> +400 if masked
      B: r, g, b
    Ref-mask 400*(1-mg) is injected by a third selector matmul.
  - Per shift: DVE subtract (fp32 in, bf16 out) over full haloed rows
    (1 free dim -> single-wait-capable ISA structs), DVE square (bf16 2x),
    selector matmuls reduce channels into a 32-partition PSUM slot
    (4 shifts per 128-partition PSUM bank), ScalarE exp(-50*d2) with fused
    accum_out -> per-partition partial sums; halo columns are skipped by
    the strided matmul rhs.
  - Raw engine programs with explicit semaphores: this toolchain only
    supports one embedded sync-wait per instruction, so every wait is its
    own wait_ge instruction (TileContext emits multi-wait instructions and
    cannot compile here).
"""

import sys

for _p in ("/opt/trn_rl_repo", "/opt/pypackages"):
    if _p not in sys.path:
        sys.path.insert(0, _p)

from contextlib import ExitStack

import numpy as np
import ml_dtypes

import concourse.bass as bass
import concourse.mybir as mybir
from concourse.ap import AP
from concourse.alu_op_type import AluOpType

F32 = mybir.dt.float32
BF16 = mybir.dt.bfloat16
BF_NP = ml_dtypes.bfloat16

R = 2
G = 32           # W-blocks; one shift-slot = 32 partitions (PE quadrant)
CA = 4           # tile A channels: x, y, z, query-mask
CB = 3           # tile B channels: r, g, b
SBATCH = 4       # shift slots per 128-partition PSUM bank
NPSUM = 6        # rotating PSUM banks
NSQ = 8          # rotating sq buffers
MQ_C = 20.0
MR_C = 400.0
EXP_SCALE = -50.0


class Cfg:
    def __init__(self, H=352, W=1216, HS=32):
        assert W % G == 0 and H % HS == 0
        self.H, self.W, self.HS = H, W, HS
        self.WB = W // G
        self.WBH = self.WB + 2 * R
        self.Hp = H + 2 * R
        self.NSLAB = H // HS
        self.NQ = G * self.Hp * self.WBH     # haloed plane elems
        self.QF = (HS + 2 * R) * self.WBH    # query tile free size
        self.SF = HS * self.WBH              # slab tile free size (full width)
        # row-chunks per slab: PSUM bank holds <=512 f32 per partition
        cr = max(1, 512 // self.WB)
        self.rchunks = []
        o = 0
        while o < HS:
            self.rchunks.append((o, min(cr, HS - o)))
            o += cr
        self.slots = [(t, dy, dx) for t in (0, 1)
                      for dy in range(-R, R + 1) for dx in range(-R, R + 1)]
        self.batches = [self.slots[i:i + SBATCH]
                        for i in range(0, len(self.slots), SBATCH)]
        self.NB = len(self.batches)          # 13
        self.NC = len(self.rchunks)          # units per batch
        self.UPS = self.NB * self.NC         # units per slab
        self.n_acc = self.NSLAB * self.UPS


def _apv(t_ap, p0, pcnt, free_dims, free_off=0):
    pstride = t_ap.ap[0][0]
    base = t_ap.offset + p0 * pstride + free_off
    return AP(t_ap.tensor, base, [[pstride, pcnt]] + [list(d) for d in free_dims])


def _dram_ap(handle, offset, dims):
    a = handle[:]
    return AP(a.tensor, a.offset + offset, [list(d) for d in dims])


def make_selA():
    s = np.zeros((CA * G, G), dtype=BF_NP)
    for c in range(CA):
        for g in range(G):
            s[c * G + g, g] = 1
    return s


def make_selB():
    s = np.zeros((CB * G, G), dtype=BF_NP)
    for c in range(CB):
        for g in range(G):
            s[c * G + g, g] = 1
    return s


def make_selvr():
    s = np.zeros((G, SBATCH * G), dtype=BF_NP)
    for t in range(SBATCH):
        for g in range(G):
            s[g, t * G + g] = 1
    return s


def emit(nc: bass.Bass, cfg: Cfg):
    HS, WB, WBH, Hp = cfg.HS, cfg.WB, cfg.WBH, cfg.Hp
    NQ, QF, SF = cfg.NQ, cfg.QF, cfg.SF
    NSLAB, NB, NC, UPS = cfg.NSLAB, cfg.NB, cfg.NC, cfg.UPS
    Act = mybir.ActivationFunctionType

    dp = nc.declare_dram_parameter
    # all planes in blocked+haloed geometry, flat [*, NQ] f32 (VR bf16)
    qa_d = dp("qa_d", [2, CA, NQ], F32, isOutput=False)   # query xyz+Vq per term
    ra_d = dp("ra_d", [2, CA, NQ], F32, isOutput=False)   # ref xyz+0 per term
    qb_d = dp("qb_d", [CB, NQ], F32, isOutput=False)      # query rgb (frame b)
    rbt_d = dp("rbt_d", [CB, NQ], F32, isOutput=False)    # ref rgb (frame tb)
    vr_d = dp("vr_d", [2, NQ], BF16, isOutput=False)      # 400*(1-mg) per term
    selA_d = dp("selA_d", [CA * G, G], BF16, isOutput=False)
    selB_d = dp("selB_d", [CB * G, G], BF16, isOutput=False)
    selvr_d = dp("selvr_d", [G, SBATCH * G], BF16, isOutput=False)
    out_d = dp("out_d", [128, 1], F32, isOutput=True)
    dbg_d = dp("dbg_d", [128, cfg.n_acc], F32, isOutput=True)

    LD = 8            # load DMAs per slab
    NCONST = 3        # constant DMAs at start

    def unit(s, b, c):
        return s * UPS + b * NC + c

    with ExitStack() as ex:
        E = ex.enter_context
        # SBUF buffers (double-buffered per slab phase)
        qa_s = [[E(nc.sbuf_tensor(f"qa{t}{p}", [CA * G, QF + 4], F32))
                 for p in range(2)] for t in range(2)]
        ra_s = [[E(nc.sbuf_tensor(f"ra{t}{p}", [CA * G, SF], F32))
                 for p in range(2)] for t in range(2)]
        qb_s = [E(nc.sbuf_tensor(f"qb{p}", [CB * G, QF + 4], F32))
                for p in range(2)]
        rbt_s = [E(nc.sbuf_tensor(f"rbt{p}", [CB * G, SF], F32))
                 for p in range(2)]
        vr_s = [[E(nc.sbuf_tensor(f"vr{t}{p}", [G, SF], BF16))
                 for p in range(2)] for t in range(2)]
        da_s = E(nc.sbuf_tensor("da", [CA * G, SF], BF16))
        db_s = E(nc.sbuf_tensor("db", [CB * G, SF], BF16))
        sqa_s = [E(nc.sbuf_tensor(f"sqa{i}", [CA * G, SF], BF16))
                 for i in range(NSQ)]
        sqb_s = [E(nc.sbuf_tensor(f"sqb{i}", [CB * G, SF], BF16))
                 for i in range(NSQ)]
        kt_s = [E(nc.sbuf_tensor(f"kt{i}", [128, 512], BF16))
                for i in range(2)]
        acc_s = E(nc.sbuf_tensor("acc", [128, cfg.n_acc], F32))
        res_s = E(nc.sbuf_tensor("res", [128, 1], F32))
        selA_s = E(nc.sbuf_tensor("selA", [CA * G, G], BF16))
        selB_s = E(nc.sbuf_tensor("selB", [CB * G, G], BF16))
        selvr_s = E(nc.sbuf_tensor("selvr", [G, SBATCH * G], BF16))
        ps_s = [E(nc.psum_tensor(f"ps{i}", [128, 512], F32))
                for i in range(NPSUM)]

        sL = E(nc.semaphore("sL"))   # misc DMA completions (+16 each)
        sLC = E(nc.semaphore("sLC"))  # constant loads
        sL0 = E(nc.semaphore("sL0"))  # even-slab loads
        sL1 = E(nc.semaphore("sL1"))  # odd-slab loads
        sG = E(nc.semaphore("sG"))   # gpsimd memset done
        sV = E(nc.semaphore("sV"))   # DVE slots done
        sP = E(nc.semaphore("sP"))   # PE units done
        sA = E(nc.semaphore("sA"))   # ACT units done
        blk = E(nc.Block())

        @blk.gpsimd
        def _(gp):
            gp.memset(acc_s.ap(), 0.0)
            gp.memset(res_s.ap(), 0.0)
            for t in range(2):
                for p in range(2):
                    gp.memset(qa_s[t][p].ap(), 0.0)
            for p in range(2):
                gp.memset(qb_s[p].ap(), 0.0)
            gp.drain()
            gp.sem_inc(sG, 8)

        @blk.sync
        def _(sp):
            sp.dma_start(selA_s[:], selA_d[:]).then_inc(sLC, 16)
            sp.dma_start(selB_s[:], selB_d[:]).then_inc(sLC, 16)
            sp.dma_start(selvr_s[:], selvr_d[:]).then_inc(sLC, 16)
            sp.wait_ge(sG, 8)
            for s in range(NSLAB):
                ph = s % 2
                if s >= 2:
                    sp.wait_ge(sV, 50 * (s - 1))
                    sp.wait_ge(sP, UPS * (s - 1))
                r0 = s * HS
                sLs = sL0 if s % 2 == 0 else sL1
                for t in range(2):
                    sp.dma_start(
                        _apv(qa_s[t][ph].ap(), 0, CA * G, [[1, QF]], 2),
                        _dram_ap(qa_d, t * CA * NQ + r0 * WBH,
                                 [[NQ, CA], [Hp * WBH, G], [1, QF]])
                    ).then_inc(sLs, 16)
                    sp.dma_start(
                        ra_s[t][ph].ap(),
                        _dram_ap(ra_d, t * CA * NQ + (r0 + 2) * WBH,
                                 [[NQ, CA], [Hp * WBH, G], [1, SF]])
                    ).then_inc(sLs, 16)
                    sp.dma_start(
                        vr_s[t][ph].ap(),
                        _dram_ap(vr_d, t * NQ + (r0 + 2) * WBH,
                                 [[Hp * WBH, G], [1, SF]])
                    ).then_inc(sLs, 16)
                sp.dma_start(
                    _apv(qb_s[ph].ap(), 0, CB * G, [[1, QF]], 2),
                    _dram_ap(qb_d, r0 * WBH,
                             [[NQ, CB], [Hp * WBH, G], [1, QF]])
                ).then_inc(sLs, 16)
                sp.dma_start(
                    rbt_s[ph].ap(),
                    _dram_ap(rbt_d, (r0 + 2) * WBH,
                             [[NQ, CB], [Hp * WBH, G], [1, SF]])
                ).then_inc(sLs, 16)
            # final output
            sp.wait_ge(sV, 50 * NSLAB + 1)
            sp.dma_start(out_d[:], res_s.ap()).then_inc(sL, 16)
            sp.dma_start(dbg_d[:], acc_s.ap()).then_inc(sL, 16)

        @blk.vector
        def _(ve):
            J = 0
            ve.wait_ge(sLC, 16 * NCONST)
            for s in range(NSLAB):
                ph = s % 2
                sLs = sL0 if s % 2 == 0 else sL1
                ve.wait_ge(sLs, 16 * LD * (s // 2 + 1))
                for j5, (t, dy, dx) in enumerate(cfg.slots):
                    if J >= NSQ:
                        Jo = J - NSQ
                        oldb = (Jo // 50) * NB + (Jo % 50) // SBATCH
                        ve.wait_ge(sP, NC * (oldb + 1))
                    qoff = 2 + (2 + dy) * WBH + dx
                    nc.vector.tensor_tensor(
                        da_s.ap(), ra_s[t][ph].ap(),
                        _apv(qa_s[t][ph].ap(), 0, CA * G, [[1, SF]], qoff),
                        AluOpType.subtract)
                    nc.vector.tensor_mul(sqa_s[J % NSQ].ap(), da_s.ap(), da_s.ap())
                    rb_ap = (_apv(qb_s[ph].ap(), 0, CB * G, [[1, SF]], 2 + 2 * WBH)
                             if t == 0 else rbt_s[ph].ap())
                    nc.vector.tensor_tensor(
                        db_s.ap(), rb_ap,
                        _apv(qb_s[ph].ap(), 0, CB * G, [[1, SF]], qoff),
                        AluOpType.subtract)
                    nc.vector.tensor_mul(
                        sqb_s[J % NSQ].ap(), db_s.ap(), db_s.ap())
                    ve.drain()
                    ve.sem_inc(sV, 1)
                    J += 1
            # final reduction of acc columns
            ve.wait_ge(sA, NSLAB * UPS)
            nc.vector.tensor_reduce(
                res_s.ap(), acc_s.ap(), axis=mybir.AxisListType.X,
                op=AluOpType.add)
            ve.drain()
            ve.sem_inc(sV, 1)

        @blk.tensor
        def _(pe):
            pe.wait_ge(sLC, 16 * NCONST)
            for s in range(NSLAB):
                ph = s % 2
                for b, bslots in enumerate(cfg.batches):
                    for c in range(NC):
                        u = unit(s, b, c)
                        if u >= NPSUM:
                            pe.wait_ge(sA, u - NPSUM + 1)
                    runs = []
                    for j, (t, _, _) in enumerate(bslots):
                        if runs and runs[-1][0] == t:
                            runs[-1][2] += 1
                        else:
                            runs.append([t, j, 1])
                    pieces = []
                    for (t, soff, scnt) in runs:
                        x, end = soff, soff + scnt
                        while x < end:
                            for size in (4, 2, 1):
                                if x % size == 0 and x + size <= end:
                                    pieces.append((t, x, size))
                                    x += size
                                    break
                    for j, (t, dy, dx) in enumerate(bslots):
                        Jg = s * 50 + b * SBATCH + j
                        pe.wait_ge(sV, Jg + 1)
                        for c, (ro, nr) in enumerate(cfg.rchunks):
                            u = unit(s, b, c)
                            pt = ps_s[u % NPSUM]
                            cn = nr * WB
                            nc.tensor.matmul(
                                pt[G * j:G * (j + 1), :cn], selA_s[:],
                                _apv(sqa_s[Jg % NSQ].ap(), 0, CA * G,
                                     [[WBH, nr], [1, WB]], ro * WBH + 2),
                                start=True, stop=False, skip_group_check=True,
                                tile_position=(0, G * j))
                            nc.tensor.matmul(
                                pt[G * j:G * (j + 1), :cn], selB_s[:],
                                _apv(sqb_s[Jg % NSQ].ap(), 0, CB * G,
                                     [[WBH, nr], [1, WB]], ro * WBH + 2),
                                start=False, stop=False, skip_group_check=True,
                                tile_position=(0, G * j))
                    for c, (ro, nr) in enumerate(cfg.rchunks):
                        u = unit(s, b, c)
                        pt = ps_s[u % NPSUM]
                        cn = nr * WB
                        for pi, (t, soff, scnt) in enumerate(pieces):
                            mm = nc.tensor.matmul(
                                pt[G * soff:G * (soff + scnt), :cn],
                                selvr_s[:, :G * scnt],
                                _apv(vr_s[t][ph].ap(), 0, G,
                                     [[WBH, nr], [1, WB]], ro * WBH + 2),
                                start=False, stop=True, skip_group_check=True,
                                tile_position=(0, G * soff))
                            if pi == len(pieces) - 1:
                                pe.drain()
                                pe.sem_inc(sP, 1)

        @blk.scalar
        def _(ac):
            ac.wait_ge(sG, 1)
            for s in range(NSLAB):
                for b in range(NB):
                    pb = G * len(cfg.batches[b])
                    for c, (ro, nr) in enumerate(cfg.rchunks):
                        u = unit(s, b, c)
                        ac.wait_ge(sP, u + 1)
                        cn = nr * WB
                        nc.scalar.activation(
                            kt_s[u % 2][:pb, :cn], ps_s[u % NPSUM][:pb, :cn],
                            Act.Exp, scale=EXP_SCALE,
                            accum_out=acc_s[:pb, u:u + 1])
                        ac.drain()
                        ac.sem_inc(sA, 1)
    return nc


# ---------------- host side ----------------

def _block_q(plane, cfg):
    """[H, W] -> flat blocked+haloed [G*Hp*WBH], zero-padded borders."""
    p = np.zeros((cfg.Hp, cfg.W + 2 * R), dtype=np.float32)
    p[R:R + cfg.H, R:R + cfg.W] = plane
    out = np.empty((G, cfg.Hp, cfg.WBH), dtype=np.float32)
    for g in range(G):
        out[g] = p[:, g * cfg.WB:g * cfg.WB + cfg.WBH]
    return np.ascontiguousarray(out).reshape(-1)


def host_precompute(rgb, depth, depth_gt, depth_mask, depth_gt_mask,
                    xy1_grid, Ts, cfg, b):
    tb = b ^ 1
    xy1 = np.asarray(xy1_grid[b], np.float32)
    dep = np.asarray(depth[b, 0], np.float32)
    dgt_b = np.asarray(depth_gt[b, 0], np.float32)
    dgt_t = np.asarray(depth_gt[tb, 0], np.float32)
    mp = np.asarray(depth_mask[b, 0], np.float32)
    mg_b = np.asarray(depth_gt_mask[b, 0], np.float32)
    mg_t = np.asarray(depth_gt_mask[tb, 0], np.float32)

    xyz_p = xy1 * dep
    T21 = (np.linalg.inv(np.asarray(Ts[tb], np.float64)) @
           np.asarray(Ts[b], np.float64)).astype(np.float32)
    Rm, tv = T21[:3, :3], T21[:3, 3]
    txyz = np.einsum('ij,jhw->ihw', Rm, xyz_p).astype(np.float32) \
        + tv[:, None, None].astype(np.float32)
    pos = (txyz[2] > 0).astype(np.float32) * mp

    qa = np.empty((2, CA, cfg.NQ), np.float32)
    ra = np.empty((2, CA, cfg.NQ), np.float32)
    for c in range(3):
        qa[0, c] = _block_q(xyz_p[c], cfg)
        qa[1, c] = _block_q(txyz[c], cfg)
        ra[0, c] = _block_q(xy1[c] * dgt_b, cfg)
        ra[1, c] = _block_q(xy1[c] * dgt_t, cfg)
    qa[0, 3] = MQ_C * (1.0 - _block_q(mp, cfg))
    qa[1, 3] = MQ_C * (1.0 - _block_q(pos, cfg))
    ra[:, 3] = 0.0
    qb = np.stack([_block_q(np.asarray(rgb[b, c], np.float32), cfg)
                   for c in range(3)])
    rbt = np.stack([_block_q(np.asarray(rgb[tb, c], np.float32), cfg)
                    for c in range(3)])
    vr = np.stack([MR_C * (1.0 - _block_q(mg_b, cfg)),
                   MR_C * (1.0 - _block_q(mg_t, cfg))]).astype(BF_NP)
    return {"qa_d": qa, "ra_d": ra, "qb_d": qb, "rbt_d": rbt, "vr_d": vr,
            "selA_d": make_selA(), "selB_d": make_selB(),
            "selvr_d": make_selvr()}


def make_in_maps(rgb, depth, depth_gt, depth_mask, depth_gt_mask, xy1_grid, Ts,
                 cfg, n_cores=8):
    return [host_precompute(rgb, depth, depth_gt, depth_mask, depth_gt_mask,
                            xy1_grid, Ts, cfg, b) for b in range(n_cores)]


_CACHED = {}


def _get_nc(cfg_key=(352, 1216, 32)):
    if cfg_key not in _CACHED:
        cfg = Cfg(*cfg_key)
        nc = bass.Bass()
        emit(nc, cfg)
        _CACHED[cfg_key] = (nc, cfg)
    return _CACHED[cfg_key]


def kernel(rgb, depth, depth_gt, depth_mask, depth_gt_mask, xy1_grid, Ts,
           **run_kwargs):
    from concourse.bass_utils import run_bass_kernel_spmd
    nc, cfg = _get_nc()
    maps = make_in_maps(rgb, depth, depth_gt, depth_mask, depth_gt_mask,
                        xy1_grid, Ts, cfg)
    res = run_bass_kernel_spmd(nc, maps, list(range(8)), **run_kwargs)
    total = np.float64(0.0)
    for r in res.results:
        total += np.float64(r["out_d"][:, 0].sum())
    n_gt = max(np.asarray(depth_gt_mask, np.float64).sum(), 1.0)
    loss = -total / n_gt
    kernel.last_results = res
    return np.float32(loss)



# revision 3
# speedup vs baseline: 1.2341x; 1.2341x over previous
"""C3DLoss kernel for Trainium2 — 8-core batch-parallel, raw-Bass implementation.

Per core = one batch frame b (tgt pairing partner tb = b^1):
    partial = sum over both terms (same-frame, cross-frame), all 25 shifts
              delta in [-2,2]^2, all pixels p of
        mref(p) * mq(p+delta) * exp(-50*(|xyz_r(p)-xyz_q(p+d)|^2
                                         + |rgb_r(p)-rgb_q(p+d)|^2))
    loss = -(sum of partials) / max(sum(depth_gt_mask), 1)

Device mapping (v2 — all four compute engines balanced):
  - Host pre-blocks planes into G=32 W-blocks.  Query planes keep a +-2
    halo in both dims (zero padded, [G, Hp, WBH]); ref planes are stored
    compact ([G, H, WB]).  Partitions = (channel, block); dy/dx shifts are
    2-D free-dim windows into the haloed query tiles.
  - 8 feature channels split across two 128-partition tiles:
      A: x, y, z, 20*(1-mq)   (query-mask channel; ref side 0)
      B: r, g, b, 20*(1-mg)   (ref-mask channel; query side 0)
    so the PE contraction of squared diffs accumulates
    d2 + 400*(1-mq) + 400*(1-mg); exp(-50*.) kills masked pairs exactly.
  - Engine split per (term,shift) x 32-row slab [all APs compact, 1216]:
      DVE:    subA = raA - qaA[window]   (fp32 -> bf16)
              subB = rbB - qbB[window]
      ACT:    sqA = Square(subA)         (bf16)
      GPSIMD: sqB = subB * subB          (bf16)
      PE:     selector matmuls reduce channels into 32-partition PSUM
              slots (4 shifts / 128-partition bank), chunks of <=512 cols
      ACT:    exp(-50*d2) with accum_out -> per-partition partial sums
  - Raw engine programs; semaphore updates ride on the instructions via
    then_inc (no per-slot drains); every cross-engine wait is its own
    wait_ge instruction (single-embedded-wait toolchain).
"""

import sys

for _p in ("/opt/trn_rl_repo", "/opt/pypackages"):
    if _p not in sys.path:
        sys.path.insert(0, _p)

from contextlib import ExitStack

import numpy as np
import ml_dtypes

import concourse.bass as bass
import concourse.mybir as mybir
from concourse.ap import AP
from concourse.alu_op_type import AluOpType

F32 = mybir.dt.float32
BF16 = mybir.dt.bfloat16
BF_NP = ml_dtypes.bfloat16

R = 2
G = 32           # W-blocks; one shift-slot = 32 partitions (PE quadrant)
CA = 4           # tile A channels: x, y, z, query-mask
CB = 4           # tile B channels: r, g, b, ref-mask
SBATCH = 4       # shift slots per 128-partition PSUM bank
NPSUM = 8        # rotating PSUM banks (all of PSUM)
NSQ = 10         # rotating sq buffers (per side)
NDA = 6          # rotating diff buffers (per side)
MQ_C = 20.0      # sqrt(400): query-mask channel magnitude
MR_C = 20.0      # sqrt(400): ref-mask channel magnitude
EXP_SCALE = -50.0


class Cfg:
    def __init__(self, H=352, W=1216, HS=32):
        assert W % G == 0 and H % HS == 0
        self.H, self.W, self.HS = H, W, HS
        self.WB = W // G
        self.WBH = self.WB + 2 * R
        self.Hp = H + 2 * R
        self.NSLAB = H // HS
        self.NQ = G * self.Hp * self.WBH     # haloed plane elems (query)
        self.NQC = G * H * self.WB           # compact plane elems (ref)
        self.QF = (HS + 2 * R) * self.WBH    # query tile free size
        self.SFC = HS * self.WB              # compact slab tile free size
        # flat column chunks per slab: PSUM bank holds <=512 f32/partition
        self.chunks = []
        o = 0
        while o < self.SFC:
            c = min(512, self.SFC - o)
            self.chunks.append((o, c))
            o += c
        self.NC = len(self.chunks)           # 3 (512,512,192)
        self.slots = [(t, dy, dx) for t in (0, 1)
                      for dy in range(-R, R + 1) for dx in range(-R, R + 1)]
        self.NSLOT = len(self.slots)         # 50
        self.batches = [self.slots[i:i + SBATCH]
                        for i in range(0, len(self.slots), SBATCH)]
        self.NB = len(self.batches)          # 13
        self.UPS = self.NB * self.NC         # units per slab (39)
        self.n_acc = self.NSLAB * self.UPS   # 429

    def batch_of_slot(self, Jg):
        return (Jg // self.NSLOT) * self.NB + (Jg % self.NSLOT) // SBATCH

    def slot_end(self, s, b):
        return s * self.NSLOT + min(SBATCH * (b + 1), self.NSLOT)


def _apv(t_ap, p0, pcnt, free_dims, free_off=0):
    pstride = t_ap.ap[0][0]
    base = t_ap.offset + p0 * pstride + free_off
    return AP(t_ap.tensor, base, [[pstride, pcnt]] + [list(d) for d in free_dims])


def _dram_ap(handle, offset, dims):
    a = handle[:]
    return AP(a.tensor, a.offset + offset, [list(d) for d in dims])


def make_sel():
    s = np.zeros((CA * G, G), dtype=BF_NP)
    for c in range(CA):
        for g in range(G):
            s[c * G + g, g] = 1
    return s


def emit(nc: bass.Bass, cfg: Cfg):
    HS, WB, WBH = cfg.HS, cfg.WB, cfg.WBH
    NQ, NQC, QF, SFC = cfg.NQ, cfg.NQC, cfg.QF, cfg.SFC
    NSLAB, NB, NC = cfg.NSLAB, cfg.NB, cfg.NC
    NSLOT = cfg.NSLOT
    Act = mybir.ActivationFunctionType

    dp = nc.declare_dram_parameter
    qa_d = dp("qa_d", [2, CA, NQ], F32, isOutput=False)    # query A (haloed)
    ra_d = dp("ra_d", [2, CA, NQC], F32, isOutput=False)   # ref A (compact)
    qb_d = dp("qb_d", [3, NQ], F32, isOutput=False)        # query rgb (haloed)
    rb_d = dp("rb_d", [2, CB, NQC], F32, isOutput=False)   # ref B (compact)
    sel_d = dp("sel_d", [CA * G, G], BF16, isOutput=False)
    out_d = dp("out_d", [128, 1], F32, isOutput=True)
    dbg_d = dp("dbg_d", [128, cfg.n_acc], F32, isOutput=True)

    LD = 7            # load DMAs per slab

    def unit(s, b, c):
        return s * cfg.UPS + b * NC + c

    with ExitStack() as ex:
        E = ex.enter_context
        qa_s = [[E(nc.sbuf_tensor(f"qa{t}{p}", [CA * G, QF], F32))
                 for p in range(2)] for t in range(2)]
        ra_s = [[E(nc.sbuf_tensor(f"ra{t}{p}", [CA * G, SFC], F32))
                 for p in range(2)] for t in range(2)]
        qb_s = [E(nc.sbuf_tensor(f"qb{p}", [CA * G, QF], F32))
                for p in range(2)]
        rb_s = [[E(nc.sbuf_tensor(f"rb{t}{p}", [CB * G, SFC], F32))
                 for p in range(2)] for t in range(2)]
        da_a = [E(nc.sbuf_tensor(f"daa{i}", [CA * G, SFC], BF16))
                for i in range(NDA)]
        da_b = [E(nc.sbuf_tensor(f"dab{i}", [CB * G, SFC], BF16))
                for i in range(NDA)]
        sq_a = [E(nc.sbuf_tensor(f"sqa{i}", [CA * G, SFC], BF16))
                for i in range(NSQ)]
        sq_b = [E(nc.sbuf_tensor(f"sqb{i}", [CB * G, SFC], BF16))
                for i in range(NSQ)]
        kt_s = [E(nc.sbuf_tensor(f"kt{i}", [128, 512], BF16))
                for i in range(2)]
        acc_s = E(nc.sbuf_tensor("acc", [128, cfg.n_acc], F32))
        res_s = E(nc.sbuf_tensor("res", [128, 1], F32))
        sel_s = E(nc.sbuf_tensor("sel", [CA * G, G], BF16))
        ps_s = [E(nc.psum_tensor(f"ps{i}", [128, 512], F32))
                for i in range(NPSUM)]

        sLC = E(nc.semaphore("sLC"))  # constant loads
        sL0 = E(nc.semaphore("sL0"))  # even-slab loads
        sL1 = E(nc.semaphore("sL1"))  # odd-slab loads
        sG = E(nc.semaphore("sG"))   # gpsimd memset done
        sVA = E(nc.semaphore("sVA"))  # DVE subA per slot
        sVB = E(nc.semaphore("sVB"))  # DVE subB per slot (+1 final reduce)
        sA1 = E(nc.semaphore("sA1"))  # ACT sqA per slot
        sG1 = E(nc.semaphore("sG1"))  # GPSIMD sqB per slot
        sP = E(nc.semaphore("sP"))   # PE per unit
        sA = E(nc.semaphore("sA"))   # ACT exp per unit
        sL = E(nc.semaphore("sL"))   # final output DMAs
        blk = E(nc.Block())

        @blk.gpsimd
        def _(gp):
            gp.memset(acc_s.ap(), 0.0)
            gp.memset(res_s.ap(), 0.0)
            for p in range(2):
                gp.memset(qb_s[p].ap(), 0.0)
            gp.drain()
            gp.sem_inc(sG, 8)
            # sqB loop: per batch, squares for its slots
            for s in range(NSLAB):
                for b in range(NB):
                    gp.wait_ge(sVB, cfg.slot_end(s, b))
                    # sq_b reuse: this batch's slots overwrite slots NSQ
                    # back; PE must have consumed their whole batch
                    lastJ = cfg.slot_end(s, b) - 1
                    if lastJ >= NSQ:
                        bold = cfg.batch_of_slot(lastJ - NSQ)
                        gp.wait_ge(sP, NC * (bold + 1))
                    for j in range(len(cfg.batches[b])):
                        Jg = s * NSLOT + b * SBATCH + j
                        nc.gpsimd.tensor_mul(
                            sq_b[Jg % NSQ].ap(), da_b[Jg % NDA].ap(),
                            da_b[Jg % NDA].ap()).then_inc(sG1, 1)

        @blk.sync
        def _(sp):
            sp.dma_start(sel_s[:], sel_d[:]).then_inc(sLC, 16)
            sp.wait_ge(sG, 8)
            for s in range(NSLAB):
                ph = s % 2
                if s >= 2:
                    sp.wait_ge(sVB, NSLOT * (s - 1))
                r0 = s * HS
                sLs = sL0 if s % 2 == 0 else sL1
                for t in range(2):
                    sp.dma_start(
                        qa_s[t][ph].ap(),
                        _dram_ap(qa_d, t * CA * NQ + r0 * WBH,
                                 [[NQ, CA], [cfg.Hp * WBH, G], [1, QF]])
                    ).then_inc(sLs, 16)
                    sp.dma_start(
                        ra_s[t][ph].ap(),
                        _dram_ap(ra_d, t * CA * NQC + r0 * WB,
                                 [[NQC, CA], [cfg.H * WB, G], [1, SFC]])
                    ).then_inc(sLs, 16)
                    sp.dma_start(
                        rb_s[t][ph].ap(),
                        _dram_ap(rb_d, t * CB * NQC + r0 * WB,
                                 [[NQC, CB], [cfg.H * WB, G], [1, SFC]])
                    ).then_inc(sLs, 16)
                sp.dma_start(
                    _apv(qb_s[ph].ap(), 0, 3 * G, [[1, QF]]),
                    _dram_ap(qb_d, r0 * WBH,
                             [[NQ, 3], [cfg.Hp * WBH, G], [1, QF]])
                ).then_inc(sLs, 16)
            # final output
            sp.wait_ge(sVB, NSLAB * NSLOT + 1)
            sp.dma_start(out_d[:], res_s.ap()).then_inc(sL, 16)
            sp.dma_start(dbg_d[:], acc_s.ap()).then_inc(sL, 16)

        @blk.vector
        def _(ve):
            for s in range(NSLAB):
                ph = s % 2
                sLs = sL0 if s % 2 == 0 else sL1
                ve.wait_ge(sLs, 16 * LD * (s // 2 + 1))
                for j5, (t, dy, dx) in enumerate(cfg.slots):
                    Jg = s * NSLOT + j5
                    if Jg >= NDA:
                        ve.wait_ge(sA1, Jg - NDA + 1)
                        ve.wait_ge(sG1, Jg - NDA + 1)
                    off = (R + dy) * WBH + (R + dx)
                    nc.vector.tensor_tensor(
                        da_a[Jg % NDA].ap(), ra_s[t][ph].ap(),
                        _apv(qa_s[t][ph].ap(), 0, CA * G,
                             [[WBH, HS], [1, WB]], off),
                        AluOpType.subtract).then_inc(sVA, 1)
                    nc.vector.tensor_tensor(
                        da_b[Jg % NDA].ap(), rb_s[t][ph].ap(),
                        _apv(qb_s[ph].ap(), 0, CB * G,
                             [[WBH, HS], [1, WB]], off),
                        AluOpType.subtract).then_inc(sVB, 1)
            # final reduction of acc columns
            ve.wait_ge(sA, cfg.n_acc)
            nc.vector.tensor_reduce(
                res_s.ap(), acc_s.ap(), axis=mybir.AxisListType.X,
                op=AluOpType.add).then_inc(sVB, 1)

        @blk.tensor
        def _(pe):
            pe.wait_ge(sLC, 16)
            for s in range(NSLAB):
                for b, bslots in enumerate(cfg.batches):
                    pe.wait_ge(sA1, cfg.slot_end(s, b))
                    pe.wait_ge(sG1, cfg.slot_end(s, b))
                    for c, (co, cn) in enumerate(cfg.chunks):
                        u = unit(s, b, c)
                        if u >= NPSUM:
                            pe.wait_ge(sA, u - NPSUM + 1)
                        pt = ps_s[u % NPSUM]
                        for j in range(len(bslots)):
                            Jg = s * NSLOT + b * SBATCH + j
                            last = (j == len(bslots) - 1)
                            nc.tensor.matmul(
                                pt[G * j:G * (j + 1), :cn], sel_s[:],
                                _apv(sq_a[Jg % NSQ].ap(), 0, CA * G,
                                     [[1, cn]], co),
                                start=True, stop=False, skip_group_check=True,
                                tile_position=(0, G * j))
                            mm = nc.tensor.matmul(
                                pt[G * j:G * (j + 1), :cn], sel_s[:],
                                _apv(sq_b[Jg % NSQ].ap(), 0, CB * G,
                                     [[1, cn]], co),
                                start=False, stop=True, skip_group_check=True,
                                tile_position=(0, G * j))
                            if last:
                                mm.then_inc(sP, 1)

        @blk.scalar
        def _(ac):
            ac.wait_ge(sG, 1)
            for s in range(NSLAB):
                for b in range(NB):
                    for j in range(len(cfg.batches[b])):
                        Jg = s * NSLOT + b * SBATCH + j
                        if Jg >= NSQ:
                            bold = cfg.batch_of_slot(Jg - NSQ)
                            ac.wait_ge(sP, NC * (bold + 1))
                        ac.wait_ge(sVA, Jg + 1)
                        nc.scalar.square(
                            sq_a[Jg % NSQ].ap(),
                            da_a[Jg % NDA].ap()).then_inc(sA1, 1)
                    # exps for the previous batch
                    bprev = b - 1
                    sprev = s
                    if b == 0:
                        sprev, bprev = s - 1, NB - 1
                    if sprev >= 0:
                        pb = G * len(cfg.batches[bprev])
                        for c, (co, cn) in enumerate(cfg.chunks):
                            u = unit(sprev, bprev, c)
                            ac.wait_ge(sP, u + 1)
                            nc.scalar.activation(
                                kt_s[u % 2][:pb, :cn],
                                ps_s[u % NPSUM][:pb, :cn],
                                Act.Exp, scale=EXP_SCALE,
                                accum_out=acc_s[:pb, u:u + 1]).then_inc(sA, 1)
            # trailing batch
            pb = G * len(cfg.batches[NB - 1])
            for c, (co, cn) in enumerate(cfg.chunks):
                u = unit(NSLAB - 1, NB - 1, c)
                ac.wait_ge(sP, u + 1)
                nc.scalar.activation(
                    kt_s[u % 2][:pb, :cn], ps_s[u % NPSUM][:pb, :cn],
                    Act.Exp, scale=EXP_SCALE,
                    accum_out=acc_s[:pb, u:u + 1]).then_inc(sA, 1)
    return nc


# ---------------- host side ----------------

def _block_q(plane, cfg):
    """[H, W] -> flat blocked+haloed [G*Hp*WBH], zero-padded borders."""
    p = np.zeros((cfg.Hp, cfg.W + 2 * R), dtype=np.float32)
    p[R:R + cfg.H, R:R + cfg.W] = plane
    out = np.empty((G, cfg.Hp, cfg.WBH), dtype=np.float32)
    for g in range(G):
        out[g] = p[:, g * cfg.WB:g * cfg.WB + cfg.WBH]
    return np.ascontiguousarray(out).reshape(-1)


def _block_c(plane, cfg):
    """[H, W] -> flat compact blocked [G*H*WB]."""
    out = np.asarray(plane, np.float32).reshape(cfg.H, G, cfg.WB)
    return np.ascontiguousarray(out.transpose(1, 0, 2)).reshape(-1)


def host_precompute(rgb, depth, depth_gt, depth_mask, depth_gt_mask,
                    xy1_grid, Ts, cfg, b):
    tb = b ^ 1
    xy1 = np.asarray(xy1_grid[b], np.float32)
    dep = np.asarray(depth[b, 0], np.float32)
    dgt_b = np.asarray(depth_gt[b, 0], np.float32)
    dgt_t = np.asarray(depth_gt[tb, 0], np.float32)
    mp = np.asarray(depth_mask[b, 0], np.float32)
    mg_b = np.asarray(depth_gt_mask[b, 0], np.float32)
    mg_t = np.asarray(depth_gt_mask[tb, 0], np.float32)

    xyz_p = xy1 * dep
    T21 = (np.linalg.inv(np.asarray(Ts[tb], np.float64)) @
           np.asarray(Ts[b], np.float64)).astype(np.float32)
    Rm, tv = T21[:3, :3], T21[:3, 3]
    txyz = np.einsum('ij,jhw->ihw', Rm, xyz_p).astype(np.float32) \
        + tv[:, None, None].astype(np.float32)
    pos = (txyz[2] > 0).astype(np.float32) * mp

    qa = np.empty((2, CA, cfg.NQ), np.float32)
    ra = np.empty((2, CA, cfg.NQC), np.float32)
    rb = np.empty((2, CB, cfg.NQC), np.float32)
    for c in range(3):
        qa[0, c] = _block_q(xyz_p[c], cfg)
        qa[1, c] = _block_q(txyz[c], cfg)
        ra[0, c] = _block_c(xy1[c] * dgt_b, cfg)
        ra[1, c] = _block_c(xy1[c] * dgt_t, cfg)
        rb[0, c] = _block_c(np.asarray(rgb[b, c], np.float32), cfg)
        rb[1, c] = _block_c(np.asarray(rgb[tb, c], np.float32), cfg)
    qa[0, 3] = MQ_C * (1.0 - _block_q(mp, cfg))
    qa[1, 3] = MQ_C * (1.0 - _block_q(pos, cfg))
    ra[:, 3] = 0.0
    rb[0, 3] = MR_C * (1.0 - _block_c(mg_b, cfg))
    rb[1, 3] = MR_C * (1.0 - _block_c(mg_t, cfg))
    qb = np.stack([_block_q(np.asarray(rgb[b, c], np.float32), cfg)
                   for c in range(3)])
    return {"qa_d": qa, "ra_d": ra, "qb_d": qb, "rb_d": rb,
            "sel_d": make_sel()}


def make_in_maps(rgb, depth, depth_gt, depth_mask, depth_gt_mask, xy1_grid, Ts,
                 cfg, n_cores=8):
    return [host_precompute(rgb, depth, depth_gt, depth_mask, depth_gt_mask,
                            xy1_grid, Ts, cfg, b) for b in range(n_cores)]


_CACHED = {}


def _get_nc(cfg_key=(352, 1216, 32)):
    if cfg_key not in _CACHED:
        cfg = Cfg(*cfg_key)
        nc = bass.Bass()
        emit(nc, cfg)
        _CACHED[cfg_key] = (nc, cfg)
    return _CACHED[cfg_key]


def kernel(rgb, depth, depth_gt, depth_mask, depth_gt_mask, xy1_grid, Ts,
           **run_kwargs):
    from concourse.bass_utils import run_bass_kernel_spmd
    nc, cfg = _get_nc()
    maps = make_in_maps(rgb, depth, depth_gt, depth_mask, depth_gt_mask,
                        xy1_grid, Ts, cfg)
    res = run_bass_kernel_spmd(nc, maps, list(range(8)), **run_kwargs)
    total = np.float64(0.0)
    for r in res.results:
        total += np.float64(r["out_d"][:, 0].sum())
    n_gt = max(np.asarray(depth_gt_mask, np.float64).sum(), 1.0)
    loss = -total / n_gt
    kernel.last_results = res
    return np.float32(loss)


# revision 5
# speedup vs baseline: 1.2898x; 1.0451x over previous
"""C3DLoss kernel for Trainium2 — 8-core batch-parallel, raw-Bass implementation.

Per core = one batch frame b (tgt pairing partner tb = b^1):
    partial = sum over both terms (same-frame, cross-frame), all 25 shifts
              delta in [-2,2]^2, all pixels p of
        mref(p) * mq(p+delta) * exp(-50*(|xyz_r(p)-xyz_q(p+d)|^2
                                         + |rgb_r(p)-rgb_q(p+d)|^2))
    loss = -(sum of partials) / max(sum(depth_gt_mask), 1)

Device mapping (v3 — four compute engines balanced, all-flat APs):
  - Host pre-blocks every plane into G=32 W-blocks with a +-2 halo in both
    dims (zero padded, [G, Hp, WBH]).  Partitions = (channel, block); dy/dx
    shifts are pure flat free-dim offsets (1-D APs everywhere on DVE — 2-D
    windowed APs cost ~35ns/row restart on DVE, measured).
  - 8 feature channels in two 128-partition tiles:
      A: x, y, z, 20*(1-mq)   (query-mask channel; ref side 0)
      B: r, g, b, 20*(1-mg)   (ref-mask channel; query side 0)
    so the PE contraction of squared diffs accumulates
    d2 + 400*(1-mq) + 400*(1-mg); exp(-50*.) kills masked pairs exactly.
  - Engine split per (term,shift) x 32-row slab [flat haloed free = 1344]:
      DVE:    subA = raA - qaA[off]      (fp32 -> bf16)
              subB = rbB - qbB[off]
      ACT:    sqA = Square(subA); sqB[z:] = Square(subB[z:])
      GPSIMD: sqB[:z] = subB[:z] * subB[:z]
      PE:     selector matmuls (row-chunks, halo cols skipped by strided
              rhs) reduce channels into 32-partition PSUM slots
      ACT:    exp(-50*d2) with accum_out -> per-partition partial sums
  - Raw engine programs; semaphore updates ride on instructions via
    then_inc; every cross-engine wait is its own wait_ge instruction.
"""

import sys

for _p in ("/opt/trn_rl_repo", "/opt/pypackages"):
    if _p not in sys.path:
        sys.path.insert(0, _p)

from contextlib import ExitStack

import numpy as np
import ml_dtypes

import concourse.bass as bass
import concourse.mybir as mybir
from concourse.ap import AP
from concourse.alu_op_type import AluOpType

F32 = mybir.dt.float32
BF16 = mybir.dt.bfloat16
BF_NP = ml_dtypes.bfloat16

R = 2
G = 32           # W-blocks; one shift-slot = 32 partitions (PE quadrant)
CA = 4           # tile A channels: x, y, z, query-mask
CB = 4           # tile B channels: r, g, b, ref-mask
SBATCH = 4       # shift slots per 128-partition PSUM bank
NPSUM = 8        # rotating PSUM banks (all of PSUM)
NSQ = 10         # rotating sq buffers (per side)
NDA = 6          # rotating diff buffers (per side)
ZGP = 928        # sqB columns done by GPSIMD (rest by ACT)
MQ_C = 20.0      # sqrt(400): query-mask channel magnitude
MR_C = 20.0      # sqrt(400): ref-mask channel magnitude
EXP_SCALE = -50.0


class Cfg:
    def __init__(self, H=352, W=1216, HS=32):
        assert W % G == 0 and H % HS == 0
        self.H, self.W, self.HS = H, W, HS
        self.WB = W // G
        self.WBH = self.WB + 2 * R
        self.Hp = H + 2 * R
        self.NSLAB = H // HS
        self.NQ = G * self.Hp * self.WBH     # haloed plane elems
        self.QF = (HS + 2 * R) * self.WBH    # query tile free size (1512)
        self.SF = HS * self.WBH              # slab tile free size (1344)
        # row-chunks per slab: PSUM bank holds <=512 f32 per partition
        cr = max(1, 512 // self.WB)
        self.rchunks = []
        o = 0
        while o < HS:
            self.rchunks.append((o, min(cr, HS - o)))
            o += cr
        self.NC = len(self.rchunks)          # 3 (13,13,6 rows)
        self.slots = [(t, dy, dx) for t in (0, 1)
                      for dy in range(-R, R + 1) for dx in range(-R, R + 1)]
        self.NSLOT = len(self.slots)         # 50
        self.batches = [self.slots[i:i + SBATCH]
                        for i in range(0, len(self.slots), SBATCH)]
        self.NB = len(self.batches)          # 13
        self.UPS = self.NB * self.NC         # units per slab (39)
        self.n_acc = self.NSLAB * self.UPS   # 429

    def batch_of_slot(self, Jg):
        return (Jg // self.NSLOT) * self.NB + (Jg % self.NSLOT) // SBATCH

    def slot_end(self, s, b):
        return s * self.NSLOT + min(SBATCH * (b + 1), self.NSLOT)


def _apv(t_ap, p0, pcnt, free_dims, free_off=0):
    pstride = t_ap.ap[0][0]
    base = t_ap.offset + p0 * pstride + free_off
    return AP(t_ap.tensor, base, [[pstride, pcnt]] + [list(d) for d in free_dims])


def _dram_ap(handle, offset, dims):
    a = handle[:]
    return AP(a.tensor, a.offset + offset, [list(d) for d in dims])


def make_sel():
    s = np.zeros((CA * G, G), dtype=BF_NP)
    for c in range(CA):
        for g in range(G):
            s[c * G + g, g] = 1
    return s


def emit(nc: bass.Bass, cfg: Cfg):
    HS, WB, WBH = cfg.HS, cfg.WB, cfg.WBH
    NQ, QF, SF = cfg.NQ, cfg.QF, cfg.SF
    NSLAB, NB, NC = cfg.NSLAB, cfg.NB, cfg.NC
    NSLOT = cfg.NSLOT
    Act = mybir.ActivationFunctionType

    dp = nc.declare_dram_parameter
    qa_d = dp("qa_d", [2, CA, NQ], F32, isOutput=False)   # query A (haloed)
    ra_d = dp("ra_d", [2, CA, NQ], F32, isOutput=False)   # ref A (haloed)
    qb_d = dp("qb_d", [3, NQ], F32, isOutput=False)       # query rgb (haloed)
    rb_d = dp("rb_d", [2, CB, NQ], F32, isOutput=False)   # ref B (haloed)
    sel_d = dp("sel_d", [CA * G, G], BF16, isOutput=False)
    out_d = dp("out_d", [128, 1], F32, isOutput=True)
    dbg_d = dp("dbg_d", [128, cfg.n_acc], F32, isOutput=True)

    LD = 7            # load DMAs per slab

    def unit(s, b, c):
        return s * cfg.UPS + b * NC + c

    with ExitStack() as ex:
        E = ex.enter_context
        qa_s = [[E(nc.sbuf_tensor(f"qa{t}{p}", [CA * G, QF + 4], F32))
                 for p in range(2)] for t in range(2)]
        ra_s = [[E(nc.sbuf_tensor(f"ra{t}{p}", [CA * G, SF], F32))
                 for p in range(2)] for t in range(2)]
        qb_s = [E(nc.sbuf_tensor(f"qb{p}", [CA * G, QF + 4], F32))
                for p in range(2)]
        rb_s = [[E(nc.sbuf_tensor(f"rb{t}{p}", [CB * G, SF], F32))
                 for p in range(2)] for t in range(2)]
        da_a = [E(nc.sbuf_tensor(f"daa{i}", [CA * G, SF], BF16))
                for i in range(NDA)]
        da_b = [E(nc.sbuf_tensor(f"dab{i}", [CB * G, SF], BF16))
                for i in range(NDA)]
        sq_a = [E(nc.sbuf_tensor(f"sqa{i}", [CA * G, SF], BF16))
                for i in range(NSQ)]
        sq_b = [E(nc.sbuf_tensor(f"sqb{i}", [CB * G, SF], BF16))
                for i in range(NSQ)]
        kt_s = [E(nc.sbuf_tensor(f"kt{i}", [128, 512], BF16))
                for i in range(2)]
        acc_s = E(nc.sbuf_tensor("acc", [128, cfg.n_acc], F32))
        res_s = E(nc.sbuf_tensor("res", [128, 1], F32))
        sel_s = E(nc.sbuf_tensor("sel", [CA * G, G], BF16))
        ps_s = [E(nc.psum_tensor(f"ps{i}", [128, 512], F32))
                for i in range(NPSUM)]

        sLC = E(nc.semaphore("sLC"))  # constant loads
        sL0 = E(nc.semaphore("sL0"))  # even-slab loads
        sL1 = E(nc.semaphore("sL1"))  # odd-slab loads
        sG = E(nc.semaphore("sG"))   # gpsimd memset done
        sVA = E(nc.semaphore("sVA"))  # DVE subA per slot
        sVB = E(nc.semaphore("sVB"))  # DVE subB per slot (+1 final reduce)
        sA1 = E(nc.semaphore("sA1"))  # ACT sqA+sqBpart per slot (2/slot)
        sG1 = E(nc.semaphore("sG1"))  # GPSIMD sqB part per slot
        sP = E(nc.semaphore("sP"))   # PE per unit
        sA = E(nc.semaphore("sA"))   # ACT exp per unit
        sL = E(nc.semaphore("sL"))   # final output DMAs
        blk = E(nc.Block())

        @blk.gpsimd
        def _(gp):
            gp.memset(acc_s.ap(), 0.0)
            gp.memset(res_s.ap(), 0.0)
            for t in range(2):
                for p in range(2):
                    gp.memset(qa_s[t][p].ap(), 0.0)
            for p in range(2):
                gp.memset(qb_s[p].ap(), 0.0)
            gp.drain()
            gp.sem_inc(sG, 8)
            # sqB[:ZGP] per batch
            for s in range(NSLAB):
                for b in range(NB):
                    gp.wait_ge(sVB, cfg.slot_end(s, b))
                    lastJ = cfg.slot_end(s, b) - 1
                    if lastJ >= NSQ:
                        bold = cfg.batch_of_slot(lastJ - NSQ)
                        gp.wait_ge(sP, NC * (bold + 1))
                    for j in range(len(cfg.batches[b])):
                        Jg = s * NSLOT + b * SBATCH + j
                        nc.gpsimd.tensor_mul(
                            _apv(sq_b[Jg % NSQ].ap(), 0, CB * G, [[1, ZGP]]),
                            _apv(da_b[Jg % NDA].ap(), 0, CB * G, [[1, ZGP]]),
                            _apv(da_b[Jg % NDA].ap(), 0, CB * G, [[1, ZGP]]),
                        ).then_inc(sG1, 1)

        @blk.sync
        def _(sp):
            sp.dma_start(sel_s[:], sel_d[:]).then_inc(sLC, 16)
            sp.wait_ge(sG, 8)
            for s in range(NSLAB):
                ph = s % 2
                if s >= 2:
                    sp.wait_ge(sVB, NSLOT * (s - 1))
                r0 = s * HS
                sLs = sL0 if s % 2 == 0 else sL1
                for t in range(2):
                    sp.dma_start(
                        _apv(qa_s[t][ph].ap(), 0, CA * G, [[1, QF]], 2),
                        _dram_ap(qa_d, t * CA * NQ + r0 * WBH,
                                 [[NQ, CA], [cfg.Hp * WBH, G], [1, QF]])
                    ).then_inc(sLs, 16)
                    sp.dma_start(
                        ra_s[t][ph].ap(),
                        _dram_ap(ra_d, t * CA * NQ + (r0 + R) * WBH,
                                 [[NQ, CA], [cfg.Hp * WBH, G], [1, SF]])
                    ).then_inc(sLs, 16)
                    sp.dma_start(
                        rb_s[t][ph].ap(),
                        _dram_ap(rb_d, t * CB * NQ + (r0 + R) * WBH,
                                 [[NQ, CB], [cfg.Hp * WBH, G], [1, SF]])
                    ).then_inc(sLs, 16)
                sp.dma_start(
                    _apv(qb_s[ph].ap(), 0, 3 * G, [[1, QF]], 2),
                    _dram_ap(qb_d, r0 * WBH,
                             [[NQ, 3], [cfg.Hp * WBH, G], [1, QF]])
                ).then_inc(sLs, 16)
            # final output
            sp.wait_ge(sVB, NSLAB * NSLOT + 1)
            sp.dma_start(out_d[:], res_s.ap()).then_inc(sL, 16)
            sp.dma_start(dbg_d[:], acc_s.ap()).then_inc(sL, 16)

        @blk.vector
        def _(ve):
            for s in range(NSLAB):
                ph = s % 2
                sLs = sL0 if s % 2 == 0 else sL1
                ve.wait_ge(sLs, 16 * LD * (s // 2 + 1))
                for j5, (t, dy, dx) in enumerate(cfg.slots):
                    Jg = s * NSLOT + j5
                    if Jg >= NDA:
                        ve.wait_ge(sA1, 2 * (Jg - NDA + 1))
                        ve.wait_ge(sG1, Jg - NDA + 1)
                    off = 2 + (R + dy) * WBH + dx
                    nc.vector.tensor_tensor(
                        da_a[Jg % NDA].ap(), ra_s[t][ph].ap(),
                        _apv(qa_s[t][ph].ap(), 0, CA * G, [[1, SF]], off),
                        AluOpType.subtract).then_inc(sVA, 1)
                    nc.vector.tensor_tensor(
                        da_b[Jg % NDA].ap(), rb_s[t][ph].ap(),
                        _apv(qb_s[ph].ap(), 0, CB * G, [[1, SF]], off),
                        AluOpType.subtract).then_inc(sVB, 1)
            # final reduction of acc columns
            ve.wait_ge(sA, cfg.n_acc)
            nc.vector.tensor_reduce(
                res_s.ap(), acc_s.ap(), axis=mybir.AxisListType.X,
                op=AluOpType.add).then_inc(sVB, 1)

        @blk.tensor
        def _(pe):
            pe.wait_ge(sLC, 16)
            for s in range(NSLAB):
                for b, bslots in enumerate(cfg.batches):
                    pe.wait_ge(sA1, 2 * cfg.slot_end(s, b))
                    pe.wait_ge(sG1, cfg.slot_end(s, b))
                    for c, (ro, nr) in enumerate(cfg.rchunks):
                        u = unit(s, b, c)
                        if u >= NPSUM:
                            pe.wait_ge(sA, u - NPSUM + 1)
                        pt = ps_s[u % NPSUM]
                        cn = nr * WB
                        for j in range(len(bslots)):
                            Jg = s * NSLOT + b * SBATCH + j
                            last = (j == len(bslots) - 1)
                            nc.tensor.matmul(
                                pt[G * j:G * (j + 1), :cn], sel_s[:],
                                _apv(sq_a[Jg % NSQ].ap(), 0, CA * G,
                                     [[WBH, nr], [1, WB]], ro * WBH + R),
                                start=True, stop=False, skip_group_check=True,
                                tile_position=(0, G * j))
                            mm = nc.tensor.matmul(
                                pt[G * j:G * (j + 1), :cn], sel_s[:],
                                _apv(sq_b[Jg % NSQ].ap(), 0, CB * G,
                                     [[WBH, nr], [1, WB]], ro * WBH + R),
                                start=False, stop=True, skip_group_check=True,
                                tile_position=(0, G * j))
                            if last:
                                mm.then_inc(sP, 1)

        @blk.scalar
        def _(ac):
            ac.wait_ge(sG, 1)
            for s in range(NSLAB):
                for b in range(NB):
                    for j in range(len(cfg.batches[b])):
                        Jg = s * NSLOT + b * SBATCH + j
                        if Jg >= NSQ:
                            bold = cfg.batch_of_slot(Jg - NSQ)
                            ac.wait_ge(sP, NC * (bold + 1))
                        ac.wait_ge(sVA, Jg + 1)
                        nc.scalar.square(
                            sq_a[Jg % NSQ].ap(),
                            da_a[Jg % NDA].ap()).then_inc(sA1, 1)
                        ac.wait_ge(sVB, Jg + 1)
                        nc.scalar.square(
                            _apv(sq_b[Jg % NSQ].ap(), 0, CB * G,
                                 [[1, SF - ZGP]], ZGP),
                            _apv(da_b[Jg % NDA].ap(), 0, CB * G,
                                 [[1, SF - ZGP]], ZGP)).then_inc(sA1, 1)
                    # exps for the previous batch
                    bprev = b - 1
                    sprev = s
                    if b == 0:
                        sprev, bprev = s - 1, NB - 1
                    if sprev >= 0:
                        pb = G * len(cfg.batches[bprev])
                        for c, (ro, nr) in enumerate(cfg.rchunks):
                            u = unit(sprev, bprev, c)
                            cn = nr * WB
                            ac.wait_ge(sP, u + 1)
                            nc.scalar.activation(
                                kt_s[u % 2][:pb, :cn],
                                ps_s[u % NPSUM][:pb, :cn],
                                Act.Exp, scale=EXP_SCALE,
                                accum_out=acc_s[:pb, u:u + 1]).then_inc(sA, 1)
            # trailing batch
            pb = G * len(cfg.batches[NB - 1])
            for c, (ro, nr) in enumerate(cfg.rchunks):
                u = unit(NSLAB - 1, NB - 1, c)
                cn = nr * WB
                ac.wait_ge(sP, u + 1)
                nc.scalar.activation(
                    kt_s[u % 2][:pb, :cn], ps_s[u % NPSUM][:pb, :cn],
                    Act.Exp, scale=EXP_SCALE,
                    accum_out=acc_s[:pb, u:u + 1]).then_inc(sA, 1)
    return nc


# ---------------- host side ----------------

def _block_q(plane, cfg):
    """[H, W] -> flat blocked+haloed [G*Hp*WBH], zero-padded borders."""
    p = np.zeros((cfg.Hp, cfg.W + 2 * R), dtype=np.float32)
    p[R:R + cfg.H, R:R + cfg.W] = plane
    out = np.empty((G, cfg.Hp, cfg.WBH), dtype=np.float32)
    for g in range(G):
        out[g] = p[:, g * cfg.WB:g * cfg.WB + cfg.WBH]
    return np.ascontiguousarray(out).reshape(-1)


def host_precompute(rgb, depth, depth_gt, depth_mask, depth_gt_mask,
                    xy1_grid, Ts, cfg, b):
    tb = b ^ 1
    xy1 = np.asarray(xy1_grid[b], np.float32)
    dep = np.asarray(depth[b, 0], np.float32)
    dgt_b = np.asarray(depth_gt[b, 0], np.float32)
    dgt_t = np.asarray(depth_gt[tb, 0], np.float32)
    mp = np.asarray(depth_mask[b, 0], np.float32)
    mg_b = np.asarray(depth_gt_mask[b, 0], np.float32)
    mg_t = np.asarray(depth_gt_mask[tb, 0], np.float32)

    xyz_p = xy1 * dep
    T21 = (np.linalg.inv(np.asarray(Ts[tb], np.float64)) @
           np.asarray(Ts[b], np.float64)).astype(np.float32)
    Rm, tv = T21[:3, :3], T21[:3, 3]
    txyz = np.einsum('ij,jhw->ihw', Rm, xyz_p).astype(np.float32) \
        + tv[:, None, None].astype(np.float32)
    pos = (txyz[2] > 0).astype(np.float32) * mp

    qa = np.empty((2, CA, cfg.NQ), np.float32)
    ra = np.empty((2, CA, cfg.NQ), np.float32)
    rb = np.empty((2, CB, cfg.NQ), np.float32)
    for c in range(3):
        qa[0, c] = _block_q(xyz_p[c], cfg)
        qa[1, c] = _block_q(txyz[c], cfg)
        ra[0, c] = _block_q(xy1[c] * dgt_b, cfg)
        ra[1, c] = _block_q(xy1[c] * dgt_t, cfg)
        rb[0, c] = _block_q(np.asarray(rgb[b, c], np.float32), cfg)
        rb[1, c] = _block_q(np.asarray(rgb[tb, c], np.float32), cfg)
    qa[0, 3] = MQ_C * (1.0 - _block_q(mp, cfg))
    qa[1, 3] = MQ_C * (1.0 - _block_q(pos, cfg))
    ra[:, 3] = 0.0
    rb[0, 3] = MR_C * (1.0 - _block_q(mg_b, cfg))
    rb[1, 3] = MR_C * (1.0 - _block_q(mg_t, cfg))
    qb = np.stack([_block_q(np.asarray(rgb[b, c], np.float32), cfg)
                   for c in range(3)])
    return {"qa_d": qa, "ra_d": ra, "qb_d": qb, "rb_d": rb,
            "sel_d": make_sel()}


def make_in_maps(rgb, depth, depth_gt, depth_mask, depth_gt_mask, xy1_grid, Ts,
                 cfg, n_cores=8):
    return [host_precompute(rgb, depth, depth_gt, depth_mask, depth_gt_mask,
                            xy1_grid, Ts, cfg, b) for b in range(n_cores)]


_CACHED = {}


def _get_nc(cfg_key=(352, 1216, 32)):
    if cfg_key not in _CACHED:
        cfg = Cfg(*cfg_key)
        nc = bass.Bass()
        emit(nc, cfg)
        _CACHED[cfg_key] = (nc, cfg)
    return _CACHED[cfg_key]


def kernel(rgb, depth, depth_gt, depth_mask, depth_gt_mask, xy1_grid, Ts,
           **run_kwargs):
    from concourse.bass_utils import run_bass_kernel_spmd
    nc, cfg = _get_nc()
    maps = make_in_maps(rgb, depth, depth_gt, depth_mask, depth_gt_mask,
                        xy1_grid, Ts, cfg)
    res = run_bass_kernel_spmd(nc, maps, list(range(8)), **run_kwargs)
    total = np.float64(0.0)
    for r in res.results:
        total += np.float64(r["out_d"][:, 0].sum())
    n_gt = max(np.asarray(depth_gt_mask, np.float64).sum(), 1.0)
    loss = -total / n_gt
    kernel.last_results = res
    return np.float32(loss)
